# revision 1
# baseline (speedup 1.0000x reference)
"""ContraNorm kernel for 8 Trainium2 NeuronCores.

Math (reference):
    norm_x = x / max(||x||_row, eps)
    sim    = (norm_x @ norm_x.T) / tau          # [N, N], tau = 1
    sim[edge_index[0], edge_index[1]] = -inf
    attn   = softmax(sim, axis=1)
    out    = 1.1 * x - 0.1 * (attn @ x)

Sharding: row-parallel, flash-attention style.  Core k owns output rows
[k*1024, (k+1)*1024).  Each core receives the FULL x but row-rolled so that
its own 1024 rows sit at positions 0:1024 — that makes the program identical
on every core (pure SPMD, no partition-id); only the input data differs.

Because sim entries are cosine similarities in [-1, 1], softmax needs no
running-max: exp(sim) is in [e^-1, e].  The -inf edge mask becomes an exact
multiply by a {0, 1} mask applied to exp(sim).  The row-sum comes for free
from a ones-column appended to the V-matmul rhs.

Per-core device pipeline (c = key/source index, m = this core's 1024 rows):
  preamble: ssq per row -> sqrt -> 1/x;  norm_x bf16; PE-transpose into
            nxT [d, c] layout; x (+ones col) as V rhs in [c, d+1] layout.
  main, for each m-half (512 cols) and c-chunk (128 rows):
      psum_sim[c,m] = nxT_d0.T@nxT + nxT_d1.T@nxT     (2 bf16 matmuls)
      expT = exp(psum_sim)     (ScalarE, PSUM->SBUF bf16)
      expT *= maskT tile       (DVE, bf16 2x mode)
      psum_V[m, 0:257] += expT_chunk.T @ [x | 1]      (4 bf16 matmuls, accum)
  epilogue per 128-row m-chunk:
      S = psum_V[:, 256]; out = (1+s)*x_own - s * psum_V[:, 0:256]/S
"""

import numpy as np
import ml_dtypes

N = 8192          # rows of x
D = 256           # features
P = 128           # SBUF partitions
NT = N // P       # 64 c-chunks
R = N // 8        # 1024 rows per core
RT = R // P       # 8 m-chunks per core
HALF = 512        # m columns per pass
SCALE = 0.1
NCORES = 8
GB = 8            # c-chunks batched per DMA (1 MiB loads)

_prog_cache = {}


def _build_program(nreps=1, mask_split=True):
    import concourse.bacc as bacc
    import concourse.tile as tile
    from concourse import mybir
    from concourse.masks import make_identity
    from contextlib import ExitStack

    f32 = mybir.dt.float32
    bf16 = mybir.dt.bfloat16
    AX = mybir.AxisListType.X
    MUL = mybir.AluOpType.mult
    Exp = mybir.ActivationFunctionType.Exp
    Sqrt = mybir.ActivationFunctionType.Sqrt

    nc = bacc.Bacc("TRN2", target_bir_lowering=False, debug=False)

    xb_h = nc.dram_tensor("xb", [N, D], bf16, kind="ExternalInput")
    xo_h = nc.dram_tensor("xo", [R, D], f32, kind="ExternalInput")
    mk_h = nc.dram_tensor("maskT", [N, R], bf16, kind="ExternalInput")
    out_h = nc.dram_tensor("out", [R, D], f32, kind="ExternalOutput")

    xb = xb_h.ap().rearrange("(t p) d -> p t d", p=P)      # [128, 64, 256]
    xo_d = xo_h.ap().rearrange("(j p) d -> p j d", p=P)    # [128, 8, 256]
    mk = mk_h.ap().rearrange("(t p) m -> p t m", p=P)      # [128, 64, 1024]
    out_d = out_h.ap()

    with ExitStack() as ctx:
        tc = ctx.enter_context(tile.TileContext(nc))

        consts = ctx.enter_context(tc.tile_pool(name="consts", bufs=1))
        pre = ctx.enter_context(tc.tile_pool(name="pre", bufs=3))
        work = ctx.enter_context(tc.tile_pool(name="work", bufs=4))
        ps_t = ctx.enter_context(tc.tile_pool(name="ps_t", bufs=2, space="PSUM"))
        ps_s = ctx.enter_context(tc.tile_pool(name="ps_s", bufs=2, space="PSUM"))
        ps_v = ctx.enter_context(tc.tile_pool(name="ps_v", bufs=1, space="PSUM"))

        xa = consts.tile([P, NT, D + 1], bf16)    # V rhs: [x | 1] per c-chunk
        nxT = consts.tile([P, 2, N], bf16)        # norm_x transposed [d, c]
        xo = consts.tile([P, RT, D], f32)         # own rows, f32, for epilogue
        ident = consts.tile([P, P], bf16)
        ssq = consts.tile([P, NT], f32)
        inv = consts.tile([P, NT], f32)

        make_identity(nc, ident)
        nc.sync.dma_start(out=xo, in_=xo_d)
        nc.gpsimd.memset(xa[:, :, D : D + 1], 1.0)

        for _rep in range(nreps):
            _emit_body(nc, tile, mybir, pre, work, ps_t, ps_s, ps_v,
                       xa, nxT, xo, ident, ssq, inv, xb, mk, out_d,
                       mask_split)

    nc.compile()
    return nc


def _emit_body(nc, tile, mybir, pre, work, ps_t, ps_s, ps_v,
               xa, nxT, xo, ident, ssq, inv, xb, mk, out_d, mask_split):
    f32 = mybir.dt.float32
    bf16 = mybir.dt.bfloat16
    MUL = mybir.AluOpType.mult
    Exp = mybir.ActivationFunctionType.Exp
    Sqrt = mybir.ActivationFunctionType.Sqrt

    if True:
        # ---- preamble: row norms + transposed normalized x ----
        for g in range(NT // GB):
            sl = slice(g * GB, (g + 1) * GB)
            # straight into the V rhs layout (strided dst, no extra copy)
            nc.sync.dma_start(out=xa[:, sl, 0:D], in_=xb[:, sl, :])
            for j in range(GB):
                t = g * GB + j
                sq = pre.tile([P, D], bf16, tag="sq")
                nc.vector.scalar_tensor_tensor(
                    out=sq,
                    in0=xa[:, t, 0:D],
                    scalar=1.0,
                    in1=xa[:, t, 0:D],
                    op0=MUL,
                    op1=MUL,
                    accum_out=ssq[:, t : t + 1],
                )
            std = pre.tile([P, GB], f32, tag="std")
            nc.scalar.activation(std, ssq[:, sl], Sqrt)
            nc.vector.reciprocal(inv[:, sl], std)
            nx8 = pre.tile([P, GB, D], bf16, tag="nx8")
            for j in range(GB):
                t = g * GB + j
                nc.vector.tensor_scalar_mul(
                    nx8[:, j], xa[:, t, 0:D], inv[:, t : t + 1]
                )
            for h in range(2):
                for q in range(2):
                    tp4 = ps_t.tile([P, 4, P], bf16, tag="tp4")
                    for jj in range(4):
                        nc.tensor.transpose(
                            tp4[:, jj],
                            nx8[:, q * 4 + jj, h * P : (h + 1) * P],
                            ident,
                        )
                    c0 = (g * GB + q * 4) * P
                    nc.vector.tensor_copy(
                        out=nxT[:, h, c0 : c0 + 4 * P], in_=tp4
                    )

        # ---- main: two passes over this core's 1024 rows (512 each) ----
        for half in range(2):
            m0 = half * HALF
            pv = [
                ps_v.tile([P, D + 1], f32, tag=f"pv{i}", name=f"pv{i}")
                for i in range(4)
            ]
            for g in range(NT // GB):
                mk8 = work.tile([P, GB, HALF], bf16, tag="mk8")
                nc.sync.dma_start(
                    out=mk8, in_=mk[:, g * GB : (g + 1) * GB, m0 : m0 + HALF]
                )
                for j in range(GB):
                    t = g * GB + j
                    ps = ps_s.tile([P, HALF], f32, tag="ps")
                    nc.tensor.matmul(
                        ps,
                        nxT[:, 0, t * P : (t + 1) * P],
                        nxT[:, 0, m0 : m0 + HALF],
                        start=True,
                        stop=False,
                    )
                    nc.tensor.matmul(
                        ps,
                        nxT[:, 1, t * P : (t + 1) * P],
                        nxT[:, 1, m0 : m0 + HALF],
                        start=False,
                        stop=True,
                    )
                    et = work.tile([P, HALF], bf16, tag="et", bufs=6)
                    nc.scalar.activation(et, ps, Exp)
                    # optionally split the mask multiply across DVE and GpSimd
                    eng = (
                        nc.gpsimd
                        if (mask_split and j % 2 == 1)
                        else nc.vector
                    )
                    eng.tensor_mul(et, et, mk8[:, j])
                    for mi in range(4):
                        nc.tensor.matmul(
                            pv[mi],
                            et[:, mi * P : (mi + 1) * P],
                            xa[:, t, :],
                            start=(t == 0),
                            stop=(t == NT - 1),
                        )
            for mi in range(4):
                jj = half * 4 + mi
                sinv = work.tile([P, 1], f32, tag="sinv")
                nc.vector.reciprocal(sinv, pv[mi][:, D : D + 1])
                res = work.tile([P, D], f32, tag="res")
                nc.vector.tensor_scalar(
                    out=res,
                    in0=pv[mi][:, 0:D],
                    scalar1=sinv,
                    scalar2=-SCALE,
                    op0=MUL,
                    op1=MUL,
                )
                t1 = work.tile([P, D], f32, tag="t1")
                nc.vector.tensor_scalar_mul(t1, xo[:, jj], 1.0 + SCALE)
                nc.vector.tensor_add(res, res, t1)
                nc.sync.dma_start(
                    out=out_d[jj * P : (jj + 1) * P, :], in_=res
                )


def get_program(nreps=1, mask_split=True):
    key = (nreps, mask_split)
    if key not in _prog_cache:
        _prog_cache[key] = _build_program(nreps, mask_split)
    return _prog_cache[key]


def make_in_maps(x, edge_index):
    x = np.asarray(x, dtype=np.float32)
    ei = np.asarray(edge_index)
    r = ei[0].astype(np.int64)
    c = ei[1].astype(np.int64)
    in_maps = []
    for k in range(NCORES):
        lo = k * R
        xb = np.roll(x, -lo, axis=0).astype(ml_dtypes.bfloat16)
        xo = np.ascontiguousarray(x[lo : lo + R])
        sel = (r >= lo) & (r < lo + R)
        m_local = r[sel] - lo
        c_rolled = (c[sel] - lo) % N
        mask = np.ones((N, R), dtype=ml_dtypes.bfloat16)
        mask[c_rolled, m_local] = 0
        in_maps.append({"xb": xb, "xo": xo, "maskT": mask})
    return in_maps


def run(x, edge_index, trace=False):
    from concourse.bass_utils import run_bass_kernel_spmd

    nc = get_program()
    in_maps = make_in_maps(x, edge_index)
    br = run_bass_kernel_spmd(nc, in_maps, list(range(NCORES)), trace=trace)
    out = np.concatenate(
        [br.results[k]["out"] for k in range(NCORES)], axis=0
    ).astype(np.float32)
    return out, br


def kernel(x, edge_index):
    out, _ = run(x, edge_index, trace=False)
    return out



# revision 7
# speedup vs baseline: 1.6592x; 1.6592x over previous
"""ContraNorm kernel for 8 Trainium2 NeuronCores — fp8 DoubleRow pipeline.

Math (reference):
    norm_x = x / max(||x||_row, eps)
    sim    = (norm_x @ norm_x.T) / tau          # [N, N], tau = 1
    sim[edge_index[0], edge_index[1]] = -inf
    attn   = softmax(sim, axis=1)
    out    = 1.1 * x - 0.1 * (attn @ x)

Sharding: row-parallel.  Core k owns output rows [k*1024, (k+1)*1024).
Each core receives inputs row-rolled so its own rows sit at c-positions
0:1024 — the program is identical on every core (pure SPMD).

Since sim is a cosine similarity in [-1, 1], softmax needs no running
max: exp(sim) is in [e^-1, e].  The -inf edge mask becomes an exact
multiply of exp(sim) by {0, 1}, applied as an integer byte multiply on
the fp8 bit patterns.  The row-sum comes from a ones-column appended to
the V-matmul rhs.

fp8 (e4m3) everywhere on the matmul paths, with DoubleRow perf mode:
  sim:  psum[c,m] = sum_{kt,dp} xt[dp,kt,c] * xt[dp,kt,m]   1 MM / c-chunk
  V:    pv[m,:]  += sum_{kt,cp} et2[cp,kt,m] * xa[cp,kt,:]  4 MM / c-pair
norm_x is pre-scaled by 16 on the host (entries ~N(0,1) in fp8); the
exp activation rescales by 1/256.

The edge mask ships bit-packed (1 MiB/core) and is expanded on-chip to
{0,1} bytes with fused (x >> b) & 0x01010101 ops on u32 lanes (DVE-only:
the Pool engine has no integer/bitwise ops).  Interpreted as fp8 those
bytes are {0, 2^-9}, so the mask is applied as a float multiply; exp is
pre-scaled by 64 (bias=ln 64 in the activation) so the kept values
land in the fp8 normal range ([23.5, 174] -> x2^-9 -> [0.046, 0.34],
an exact exponent shift).  The x2^-9 cancels in the softmax ratio.

Per-core inputs (6.07 MiB vs 21 MiB for the dense-bf16-mask variant):
  xt   [128, 2, 8192] fp8   16*norm_x rolled, transposed
  xa   [128, 32, 2, 257] fp8  x rolled (V rhs layout) + ones column
  bits [128, 64, 2, 16] u32  keep-mask bits, b*64+j column mapping
  xo   [1024, 256] f32      own rows for the 1.1*x epilogue term
"""

import numpy as np
import ml_dtypes

N = 8192          # rows of x
D = 256           # features
P = 128           # SBUF partitions
NT = N // P       # 64 c-chunks
R = N // 8        # 1024 rows per core
HALF = 512        # m columns per pass
NPAIR = NT // 2   # 32 c-chunk pairs
SCALE = 0.1
NCORES = 8

_prog_cache = {}


def _build_program():
    import concourse.bacc as bacc
    import concourse.tile as tile
    from concourse import mybir
    from contextlib import ExitStack

    f32 = mybir.dt.float32
    fp8 = mybir.dt.float8e4
    u32 = mybir.dt.uint32
    u8 = mybir.dt.uint8
    DR = mybir.MatmulPerfMode.DoubleRow
    Exp = mybir.ActivationFunctionType.Exp
    SHR = mybir.AluOpType.logical_shift_right
    AND = mybir.AluOpType.bitwise_and
    MUL = mybir.AluOpType.mult
    ADD = mybir.AluOpType.add

    nc = bacc.Bacc("TRN2", target_bir_lowering=False, debug=False)

    xt_h = nc.dram_tensor("xt", [P, 2, N], fp8, kind="ExternalInput")
    xa_h = nc.dram_tensor("xa", [P, NPAIR, 2, D + 1], fp8, kind="ExternalInput")
    bits_h = nc.dram_tensor("bits", [P, NT, 2, 16], u32, kind="ExternalInput")
    xo_h = nc.dram_tensor("xo", [R, D], f32, kind="ExternalInput")
    out_h = nc.dram_tensor("out", [R, D], f32, kind="ExternalOutput")

    xo_d = xo_h.ap().rearrange("(j p) d -> p j d", p=P)    # [128, 8, 256]
    out_d = out_h.ap()

    with ExitStack() as ctx:
        tc = ctx.enter_context(tile.TileContext(nc))

        consts = ctx.enter_context(tc.tile_pool(name="consts", bufs=1))
        maskp = ctx.enter_context(tc.tile_pool(name="maskp", bufs=2))
        work = ctx.enter_context(tc.tile_pool(name="work", bufs=4))
        ps_s = ctx.enter_context(tc.tile_pool(name="ps_s", bufs=2, space="PSUM"))
        ps_v = ctx.enter_context(tc.tile_pool(name="ps_v", bufs=1, space="PSUM"))

        xt = consts.tile([P, 2, N], fp8)
        xa = consts.tile([P, NPAIR, 2, D + 1], fp8)
        bits = consts.tile([P, NT, 2, 16], u32)
        xo = consts.tile([P, R // P, D], f32)
        ebias = consts.tile([P, 1], f32)
        nc.gpsimd.memset(ebias, float(np.log(64.0)))

        # bits first (mask expansion is the first compute), then the
        # matmul operands in chunks so compute starts early.
        nc.sync.dma_start(out=bits, in_=bits_h.ap())
        nc.sync.dma_start(out=xt[:, :, 0:R], in_=xt_h.ap()[:, :, 0:R])
        nc.sync.dma_start(out=xt[:, :, R:N], in_=xt_h.ap()[:, :, R:N])
        NXA = 4
        for q in range(NXA):
            sl = slice(q * (NPAIR // NXA), (q + 1) * (NPAIR // NXA))
            nc.sync.dma_start(out=xa[:, sl], in_=xa_h.ap()[:, sl])
        nc.sync.dma_start(out=xo, in_=xo_d)

        for h in range(2):
            m0 = h * HALF
            # ---- expand this half's mask bits to {0,1} bytes ----
            mexp = maskp.tile([P, NT, HALF], fp8, tag="mexp")
            mexp32 = mexp.bitcast(u32)           # [P, NT, 128]
            for b in range(8):
                nc.vector.tensor_scalar(
                    out=mexp32[:, :, b * 16 : (b + 1) * 16],
                    in0=bits[:, :, h, :],
                    scalar1=b,
                    scalar2=0x01010101,
                    op0=SHR,
                    op1=AND,
                )

            pv = [
                ps_v.tile([P, D + 1], f32, tag=f"pv{mi}", name=f"pv{mi}")
                for mi in range(4)
            ]
            for g in range(NPAIR):
                pss = ps_s.tile([P, 2, HALF], f32, tag="pss")
                for kt in range(2):
                    t = 2 * g + kt
                    nc.tensor.matmul(
                        pss[:, kt, :],
                        xt[:, :, t * P : (t + 1) * P],
                        xt[:, :, m0 : m0 + HALF],
                        start=True,
                        stop=True,
                        perf_mode=DR,
                    )
                et2 = work.tile([P, 2, HALF], fp8, tag="et2", bufs=6)
                nc.scalar.activation(
                    et2.rearrange("p a b -> p (a b)"),
                    pss.rearrange("p a b -> p (a b)"),
                    Exp,
                    scale=1.0 / 256.0,
                    bias=ebias,
                )
                # mask apply: fp8 float multiply by {0, 2^-9}
                nc.vector.tensor_tensor(
                    out=et2[:, 0, :], in0=et2[:, 0, :],
                    in1=mexp[:, 2 * g, :], op=MUL,
                )
                nc.gpsimd.tensor_tensor(
                    out=et2[:, 1, :], in0=et2[:, 1, :],
                    in1=mexp[:, 2 * g + 1, :], op=MUL,
                )
                for mi in range(4):
                    nc.tensor.matmul(
                        pv[mi],
                        et2[:, :, mi * P : (mi + 1) * P],
                        xa[:, g],
                        start=(g == 0),
                        stop=(g == NPAIR - 1),
                        perf_mode=DR,
                    )
            # ---- epilogue: out = 1.1*x - 0.1 * pv/S ----
            for mi in range(4):
                jj = h * 4 + mi
                sinv = work.tile([P, 1], f32, tag="sinv")
                nc.vector.reciprocal(sinv, pv[mi][:, D : D + 1])
                res = work.tile([P, D], f32, tag="res")
                nc.vector.tensor_scalar(
                    out=res,
                    in0=pv[mi][:, 0:D],
                    scalar1=sinv,
                    scalar2=-SCALE,
                    op0=MUL,
                    op1=MUL,
                )
                nc.vector.scalar_tensor_tensor(
                    out=res,
                    in0=xo[:, jj],
                    scalar=1.0 + SCALE,
                    in1=res,
                    op0=MUL,
                    op1=ADD,
                )
                nc.sync.dma_start(
                    out=out_d[jj * P : (jj + 1) * P, :], in_=res
                )

    nc.compile()
    return nc


def get_program():
    if "prog" not in _prog_cache:
        _prog_cache["prog"] = _build_program()
    return _prog_cache["prog"]


def make_in_maps(x, edge_index):
    fp8 = ml_dtypes.float8_e4m3
    x = np.asarray(x, dtype=np.float32)
    ei = np.asarray(edge_index)
    r = ei[0].astype(np.int64)
    c = ei[1].astype(np.int64)

    norm = np.sqrt((x * x).sum(axis=1, keepdims=True))
    nx16 = np.asarray((x / np.maximum(norm, 1e-12)) * 16.0, dtype=fp8)
    x8 = np.asarray(x, dtype=fp8)

    in_maps = []
    for k in range(NCORES):
        lo = k * R
        nxr = np.roll(nx16, -lo, axis=0)          # [N, D] fp8
        xar = np.roll(x8, -lo, axis=0)            # [N, D] fp8

        # xt[p, kt, c] = nxr[c, kt*128 + p]
        xt = np.ascontiguousarray(
            nxr.T.reshape(2, P, N).transpose(1, 0, 2)
        )
        # xa[p, g, kt, j] = xar[(2g+kt)*128 + p, j], ones at j=256
        xa = np.empty((P, NPAIR, 2, D + 1), dtype=fp8)
        xa[:, :, :, 0:D] = xar.reshape(NPAIR, 2, P, D).transpose(2, 0, 1, 3)
        xa[:, :, :, D] = fp8(1.0)

        # keep-mask, rolled: mask[c_rolled, m_local] = 0 on edges
        sel = (r >= lo) & (r < lo + R)
        m_local = (r[sel] - lo).astype(np.int64)
        c_rolled = (c[sel] - lo) % N
        mask = np.ones((N, R), dtype=np.uint8)
        mask[c_rolled, m_local] = 0
        # column mapping m = h*512 + b*64 + j  ->  byte[c, h, j] bit b
        mm = mask.reshape(N, 2, 8, 64)
        packed = np.packbits(mm, axis=2, bitorder="little")  # [N, 2, 1, 64]
        packed = packed.reshape(N, 2, 64)
        # bits[p, t, h, w] = u32 view of packed[t*128+p, h, 4w:4w+4]
        bits = (
            packed.reshape(NT, P, 2, 64)
            .transpose(1, 0, 2, 3)
            .copy()
            .view("<u4")
        )
        xo = np.ascontiguousarray(x[lo : lo + R])
        in_maps.append({"xt": xt, "xa": xa, "bits": bits, "xo": xo})
    return in_maps


def run(x, edge_index, trace=False):
    from concourse.bass_utils import run_bass_kernel_spmd

    nc = get_program()
    in_maps = make_in_maps(x, edge_index)
    br = run_bass_kernel_spmd(nc, in_maps, list(range(NCORES)), trace=trace)
    out = np.concatenate(
        [br.results[k]["out"] for k in range(NCORES)], axis=0
    ).astype(np.float32)
    return out, br


def kernel(x, edge_index):
    out, _ = run(x, edge_index, trace=False)
    return out


# revision 11
# speedup vs baseline: 1.7677x; 1.0654x over previous
"""ContraNorm kernel for 8 Trainium2 NeuronCores — fp8 DoubleRow pipeline.

Math (reference):
    norm_x = x / max(||x||_row, eps)
    sim    = (norm_x @ norm_x.T) / tau          # [N, N], tau = 1
    sim[edge_index[0], edge_index[1]] = -inf
    attn   = softmax(sim, axis=1)
    out    = 1.1 * x - 0.1 * (attn @ x)

Sharding: row-parallel.  Core k owns output rows [k*1024, (k+1)*1024).
Each core receives inputs row-rolled so its own rows sit at c-positions
0:1024 — the program is identical on every core (pure SPMD).

Since sim is a cosine similarity in [-1, 1], softmax needs no running
max: exp(sim) is in [e^-1, e].  The -inf edge mask becomes an exact
multiply of exp(sim) by {0, 1}, applied as an integer byte multiply on
the fp8 bit patterns.  The row-sum comes from a ones-column appended to
the V-matmul rhs.

fp8 (e4m3) everywhere on the matmul paths, with DoubleRow perf mode:
  sim:  psum[c,m] = sum_{kt,dp} xt[dp,kt,c] * xt[dp,kt,m]   1 MM / c-chunk
  V:    pv[m,:]  += sum_{kt,cp} et2[cp,kt,m] * xa[cp,kt,:]  4 MM / c-pair
norm_x is pre-scaled by 16 on the host (entries ~N(0,1) in fp8); the
exp activation rescales by 1/256.

The edge mask ships bit-packed (1 MiB/core) and is expanded on-chip to
{0x00, 0x08} bytes with one fused (x shift) & 0x08080808 op per bit on
u32 lanes (DVE-only: the Pool engine has no integer/bitwise ops).
Interpreted as fp8 those bytes are {0, 2^-6}, so the mask is applied
as an fp8 float multiply on either DVE or GpSimd (pairs split 2:1 so
the exp activation stays the pacer); exp is pre-scaled by 64 (bias =
ln 64) so kept values are an exact exponent shift back into [0.37,
2.72].  The uniform 2^-6 cancels in the softmax ratio.

Per-core inputs (6.07 MiB vs 21 MiB for the dense-bf16-mask variant):
  xt   [128, 2, 8192] fp8   16*norm_x rolled, transposed
  xa   [128, 32, 2, 257] fp8  x rolled (V rhs layout) + ones column
  bits [128, 64, 2, 16] u32  keep-mask bits, b*64+j column mapping
  xo   [1024, 256] f32      own rows for the 1.1*x epilogue term
"""

import numpy as np
import ml_dtypes

N = 8192          # rows of x
D = 256           # features
P = 128           # SBUF partitions
NT = N // P       # 64 c-chunks
R = N // 8        # 1024 rows per core
HALF = 512        # m columns per pass
NPAIR = NT // 2   # 32 c-chunk pairs
SCALE = 0.1
NCORES = 8

_prog_cache = {}


def _build_program():
    import concourse.bacc as bacc
    import concourse.tile as tile
    from concourse import mybir
    from contextlib import ExitStack

    f32 = mybir.dt.float32
    fp8 = mybir.dt.float8e4
    u32 = mybir.dt.uint32
    u8 = mybir.dt.uint8
    DR = mybir.MatmulPerfMode.DoubleRow
    Exp = mybir.ActivationFunctionType.Exp
    SHR = mybir.AluOpType.logical_shift_right
    SHL = mybir.AluOpType.logical_shift_left
    AND = mybir.AluOpType.bitwise_and
    MUL = mybir.AluOpType.mult
    ADD = mybir.AluOpType.add

    nc = bacc.Bacc("TRN2", target_bir_lowering=False, debug=False)

    xt_h = nc.dram_tensor("xt", [P, 2, N], fp8, kind="ExternalInput")
    xa_h = nc.dram_tensor("xa", [P, NPAIR, 2, D + 1], fp8, kind="ExternalInput")
    bits_h = nc.dram_tensor("bits", [P, NT, 2, 16], u32, kind="ExternalInput")
    xo_h = nc.dram_tensor("xo", [R, D], f32, kind="ExternalInput")
    out_h = nc.dram_tensor("out", [R, D], f32, kind="ExternalOutput")

    xo_d = xo_h.ap().rearrange("(j p) d -> p j d", p=P)    # [128, 8, 256]
    out_d = out_h.ap()

    with ExitStack() as ctx:
        tc = ctx.enter_context(tile.TileContext(nc))

        consts = ctx.enter_context(tc.tile_pool(name="consts", bufs=1))
        maskp = ctx.enter_context(tc.tile_pool(name="maskp", bufs=2))
        work = ctx.enter_context(tc.tile_pool(name="work", bufs=4))
        ps_s = ctx.enter_context(tc.tile_pool(name="ps_s", bufs=2, space="PSUM"))
        ps_v = ctx.enter_context(tc.tile_pool(name="ps_v", bufs=1, space="PSUM"))

        xt = consts.tile([P, 2, N], fp8)
        xa = consts.tile([P, NPAIR, 2, D + 1], fp8)
        bits = consts.tile([P, NT, 2, 16], u32)
        xo = consts.tile([P, R // P, D], f32)
        ebias = consts.tile([P, 1], f32)
        nc.gpsimd.memset(ebias, float(np.log(64.0)))

        # bits first (mask expansion is the first compute), then the
        # matmul operands in chunks so compute starts early.
        nc.sync.dma_start(out=bits, in_=bits_h.ap())
        nc.sync.dma_start(out=xt[:, :, 0:R], in_=xt_h.ap()[:, :, 0:R])
        nc.sync.dma_start(out=xt[:, :, R:N], in_=xt_h.ap()[:, :, R:N])
        NXA = 4
        for q in range(NXA):
            sl = slice(q * (NPAIR // NXA), (q + 1) * (NPAIR // NXA))
            nc.sync.dma_start(out=xa[:, sl], in_=xa_h.ap()[:, sl])
        nc.sync.dma_start(out=xo, in_=xo_d)

        for h in range(2):
            m0 = h * HALF
            # ---- expand this half's mask bits to {0,1} bytes ----
            mexp = maskp.tile([P, NT, HALF], fp8, tag="mexp")
            mexp32 = mexp.bitcast(u32)           # [P, NT, 128]
            for b in range(8):
                # bit b -> byte 0x08 (fp8 2^-6): shift bit b to position 3
                nc.vector.tensor_scalar(
                    out=mexp32[:, :, b * 16 : (b + 1) * 16],
                    in0=bits[:, :, h, :],
                    scalar1=(3 - b) if b < 3 else (b - 3),
                    scalar2=0x08080808,
                    op0=SHL if b < 3 else SHR,
                    op1=AND,
                )

            pv = [
                ps_v.tile([P, D + 1], f32, tag=f"pv{mi}", name=f"pv{mi}")
                for mi in range(4)
            ]
            for g in range(NPAIR):
                pss = ps_s.tile([P, 2, HALF], f32, tag="pss")
                for kt in range(2):
                    t = 2 * g + kt
                    nc.tensor.matmul(
                        pss[:, kt, :],
                        xt[:, :, t * P : (t + 1) * P],
                        xt[:, :, m0 : m0 + HALF],
                        start=True,
                        stop=True,
                        perf_mode=DR,
                    )
                et2 = work.tile([P, 2, HALF], fp8, tag="et2", bufs=6)
                nc.scalar.activation(
                    et2.rearrange("p a b -> p (a b)"),
                    pss.rearrange("p a b -> p (a b)"),
                    Exp,
                    scale=1.0 / 256.0,
                    bias=ebias,
                )
                # mask apply: fp8 float multiply by {0, 2^-6}, out-of-place
                et2m = work.tile([P, 2, HALF], fp8, tag="et2m", bufs=6)
                eng = nc.gpsimd if g % 3 == 2 else nc.vector
                eng.tensor_tensor(
                    out=et2m.rearrange("p a b -> p (a b)"),
                    in0=et2.rearrange("p a b -> p (a b)"),
                    in1=mexp[:, 2 * g : 2 * g + 2, :].rearrange(
                        "p a b -> p (a b)"
                    ),
                    op=MUL,
                )
                for mi in range(4):
                    nc.tensor.matmul(
                        pv[mi],
                        et2m[:, :, mi * P : (mi + 1) * P],
                        xa[:, g],
                        start=(g == 0),
                        stop=(g == NPAIR - 1),
                        perf_mode=DR,
                    )
            # ---- epilogue: out = 1.1*x - 0.1 * pv/S ----
            for mi in range(4):
                jj = h * 4 + mi
                sinv = work.tile([P, 1], f32, tag="sinv")
                nc.vector.reciprocal(sinv, pv[mi][:, D : D + 1])
                res = work.tile([P, D], f32, tag="res")
                nc.vector.tensor_scalar(
                    out=res,
                    in0=pv[mi][:, 0:D],
                    scalar1=sinv,
                    scalar2=-SCALE,
                    op0=MUL,
                    op1=MUL,
                )
                nc.vector.scalar_tensor_tensor(
                    out=res,
                    in0=xo[:, jj],
                    scalar=1.0 + SCALE,
                    in1=res,
                    op0=MUL,
                    op1=ADD,
                )
                nc.sync.dma_start(
                    out=out_d[jj * P : (jj + 1) * P, :], in_=res
                )

    nc.compile()
    return nc


def get_program():
    if "prog" not in _prog_cache:
        _prog_cache["prog"] = _build_program()
    return _prog_cache["prog"]


def make_in_maps(x, edge_index):
    fp8 = ml_dtypes.float8_e4m3
    x = np.asarray(x, dtype=np.float32)
    ei = np.asarray(edge_index)
    r = ei[0].astype(np.int64)
    c = ei[1].astype(np.int64)

    norm = np.sqrt((x * x).sum(axis=1, keepdims=True))
    nx16 = np.asarray((x / np.maximum(norm, 1e-12)) * 16.0, dtype=fp8)
    x8 = np.asarray(x, dtype=fp8)

    in_maps = []
    for k in range(NCORES):
        lo = k * R
        nxr = np.roll(nx16, -lo, axis=0)          # [N, D] fp8
        xar = np.roll(x8, -lo, axis=0)            # [N, D] fp8

        # xt[p, kt, c] = nxr[c, kt*128 + p]
        xt = np.ascontiguousarray(
            nxr.T.reshape(2, P, N).transpose(1, 0, 2)
        )
        # xa[p, g, kt, j] = xar[(2g+kt)*128 + p, j], ones at j=256
        xa = np.empty((P, NPAIR, 2, D + 1), dtype=fp8)
        xa[:, :, :, 0:D] = xar.reshape(NPAIR, 2, P, D).transpose(2, 0, 1, 3)
        xa[:, :, :, D] = fp8(1.0)

        # keep-mask, rolled: mask[c_rolled, m_local] = 0 on edges
        sel = (r >= lo) & (r < lo + R)
        m_local = (r[sel] - lo).astype(np.int64)
        c_rolled = (c[sel] - lo) % N
        mask = np.ones((N, R), dtype=np.uint8)
        mask[c_rolled, m_local] = 0
        # column mapping m = h*512 + b*64 + j  ->  byte[c, h, j] bit b
        mm = mask.reshape(N, 2, 8, 64)
        packed = np.packbits(mm, axis=2, bitorder="little")  # [N, 2, 1, 64]
        packed = packed.reshape(N, 2, 64)
        # bits[p, t, h, w] = u32 view of packed[t*128+p, h, 4w:4w+4]
        bits = (
            packed.reshape(NT, P, 2, 64)
            .transpose(1, 0, 2, 3)
            .copy()
            .view("<u4")
        )
        xo = np.ascontiguousarray(x[lo : lo + R])
        in_maps.append({"xt": xt, "xa": xa, "bits": bits, "xo": xo})
    return in_maps


def run(x, edge_index, trace=False):
    from concourse.bass_utils import run_bass_kernel_spmd

    nc = get_program()
    in_maps = make_in_maps(x, edge_index)
    br = run_bass_kernel_spmd(nc, in_maps, list(range(NCORES)), trace=trace)
    out = np.concatenate(
        [br.results[k]["out"] for k in range(NCORES)], axis=0
    ).astype(np.float32)
    return out, br


def kernel(x, edge_index):
    out, _ = run(x, edge_index, trace=False)
    return out
